# revision 33
# baseline (speedup 1.0000x reference)
"""MoE (top-2 of 8 experts + shared expert) Trainium2 kernel, 8 NeuronCores.

Strategy
--------
Host (numpy): router matmul + top-2 + softmax gates (0.01% of FLOPs), token
dispatch (gather by expert), final combine (concat shared slices, scatter-add
gated expert outputs).  The per-token gate is applied on the host during the
combine, so the device computes the ungated expert FFN (one copy of x, no
broadcast multiply on device).

Device (8 cores, SPMD): core c computes
  1. expert c's FFN over the tokens routed to it, capped at capacity C=1024
     (2 clean 512-token chunks).  Tokens beyond the cap (~1% for balanced
     routing) are computed on the host in fp32 during the combine.
  2. the shared-expert FFN for token slice [c*512, (c+1)*512).

All matmul inputs are fp16 (host-rounded); PSUM accumulates fp32.  fp16 runs
at the full PE rate (1 cycle/row) and halves both HBM traffic and SBUF
footprint vs fp32, leaving DMA far under the PE roofline.  Both GEMMs loop
token-chunk-outermost, re-streaming the weight tiles once per 512-token pass
(double the weight traffic, still ~2x under the DMA budget) so that only one
x chunk is needed to start and the PE never waits on DMA.  Weights are
pre-swizzled on the host into the exact SBUF tile layout so each DMA is one
fully-contiguous block; inputs/outputs ride the gpsimd(Pool) DMA queue in
parallel with weights on sync.

Everything is feature-major ("transposed": [feature, token]) so the
contraction dim is always the SBUF partition dim.
"""

import math

import numpy as np

import concourse.bass as bass
import concourse.mybir as mybir
import concourse.tile as tile
from concourse.bass_utils import run_bass_kernel_spmd

T, D, E, F, FS, TOP_K = 4096, 2048, 8, 4096, 4096, 2
NCORES = 8
P = 128
TS = T // NCORES  # shared-expert tokens per core
DK = D // P       # contraction tiles over D
FT = F // P       # f-tiles over F
DT = D // P       # output d-tiles
CAP = 1024        # device token capacity per expert; overflow done on host
CH = 512          # token chunk size (= one PSUM bank of fp32)

F16 = mybir.dt.float16
F32 = mybir.dt.float32


def _split_multiwaits(nc):
    """This toolchain's walrus allows at most ONE fused sem-wait per
    instruction, but TileContext's assign_waits can emit several. Split the
    extras into standalone InstEventSemaphore instructions inserted
    immediately before the owning instruction on the same engine."""
    for fn in nc.m.functions:
        for bb in fn.blocks:
            insts = list(bb.instructions)
            out = []
            changed = False
            for inst in insts:
                si = inst.sync_info
                waits = list(si.on_wait) if (si and si.on_wait) else []
                if len(waits) > 1:
                    for w in waits[:-1]:
                        out.append(
                            mybir.InstEventSemaphore(
                                name=nc.get_next_instruction_name(),
                                engine=inst.engine,
                                ins=[],
                                outs=[],
                                sync_info=mybir.SyncInfo(on_wait=[w], on_update=[]),
                            )
                        )
                    inst.sync_info = mybir.SyncInfo(
                        on_wait=[waits[-1]], on_update=list(si.on_update)
                    )
                    changed = True
                out.append(inst)
            if changed:
                bb.instructions = out


def _emit_g1(nc, pools, x_d, w13_d, n_tok, nch, xt_pre=None):
    """GEMM1 + SwiGLU: aT[f, t] = silu(x@w13g) * (x@w13u), f16.
    w13_d: [FT, P, 2*DK*P] (per f-tile: gate then up halves, k-major).
    x_d: [nch, P, DK, CH] f16, chunk-major so each chunk is one contiguous
    transfer on the gpsimd/Pool queue (parallel with weights on sync)."""
    xp, wp, w2p, atp, op, ps = pools
    silu = mybir.ActivationFunctionType.Silu
    if xt_pre is not None:
        xt = xt_pre
    else:
        xt = xp.tile([P, nch, DK, CH], F16, tag="x", name="xt")
        # chunk 0 gates the very first matmul: split it across BOTH DMA
        # queues (half on sync ahead of the weight stream, half on gpsimd)
        # so it lands at ~2x queue bandwidth
        hk = DK // 2
        nc.gpsimd.dma_start(out=xt[:, 0, hk:], in_=x_d[:][0, :, hk:])
        nc.sync.dma_start(out=xt[:, 0, :hk], in_=x_d[:][0, :, :hk])
    aT = atp.tile([P, FT, n_tok], F16, tag="aT", name="aT")

    # chunk-outer: pass ci re-streams all w13 tiles (weights are re-DMA'd
    # per pass — bandwidth is cheap) so only x chunk ci is needed per pass;
    # x chunk ci+1 is prefetched from mid-pass ci (~110us ahead), on the
    # gpsimd queue where it shares bandwidth only with steady-state weights.
    for ci in range(nch):
        c0 = ci * CH
        for ft in range(FT):
            if xt_pre is None and ft == FT // 2 and ci + 1 < nch:
                nc.gpsimd.dma_start(out=xt[:, ci + 1], in_=x_d[:][ci + 1])
            wt = wp.tile([P, 2 * DK * P], F16, tag="w13", name="wt")
            nc.sync.dma_start(out=wt, in_=w13_d[:][ft])
            pg = ps.tile([P, 512], F32, tag="ps", name="pg")
            for k in range(DK):
                nc.tensor.matmul(
                    pg[:],
                    wt[:, k * P : (k + 1) * P],
                    xt[:, ci, k],
                    start=(k == 0),
                    stop=(k == DK - 1),
                )
            nc.scalar.activation(
                out=aT[:, ft, c0 : c0 + CH], in_=pg[:], func=silu
            )
            pu = ps.tile([P, 512], F32, tag="ps", name="pu")
            for k in range(DK):
                nc.tensor.matmul(
                    pu[:],
                    wt[:, (DK + k) * P : (DK + k + 1) * P],
                    xt[:, ci, k],
                    start=(k == 0),
                    stop=(k == DK - 1),
                )
            sl = aT[:, ft, c0 : c0 + CH]
            nc.vector.tensor_mul(out=sl, in0=sl, in1=pu[:])
    return aT


def _emit_g2(nc, pools, aT, w2_d, out_d, n_tok, nch):
    """GEMM2: out[d, t] = aT.T-contract over F.  w2_d: [DT, P, FT*P].
    out_d: [DT, P, n_tok] f32; output DMAs go on the gpsimd queue."""
    xp, wp, w2p, atp, op, ps = pools
    for ci in range(nch):
        c0 = ci * CH
        for dt_i in range(DT):
            w2t = w2p.tile([P, FT * P], F16, tag="w2", name="w2t")
            nc.sync.dma_start(out=w2t, in_=w2_d[:][dt_i])
            ot = op.tile([P, CH], F32, tag="o", name="ot")
            py = ps.tile([P, 512], F32, tag="ps", name="py")
            for k in range(FT):
                nc.tensor.matmul(
                    py[:],
                    w2t[:, k * P : (k + 1) * P],
                    aT[:, k, c0 : c0 + CH],
                    start=(k == 0),
                    stop=(k == FT - 1),
                )
            nc.vector.tensor_copy(out=ot, in_=py[:])
            nc.gpsimd.dma_start(out=out_d[:][dt_i, :, c0 : c0 + CH], in_=ot)


def build_program(C):
    nch = C // CH
    nchs = TS // CH
    nc = bass.Bass()
    xT = nc.dram_tensor("xT", [nch, P, DK, CH], F16, kind="ExternalInput")
    w13T = nc.dram_tensor("w13T", [FT, P, 2 * DK * P], F16, kind="ExternalInput")
    w2T = nc.dram_tensor("w2T", [DT, P, FT * P], F16, kind="ExternalInput")
    xsT = nc.dram_tensor("xsT", [nchs, P, DK, CH], F16, kind="ExternalInput")
    sw13T = nc.dram_tensor("sw13T", [FT, P, 2 * DK * P], F16, kind="ExternalInput")
    sw2T = nc.dram_tensor("sw2T", [DT, P, FT * P], F16, kind="ExternalInput")
    yeT = nc.dram_tensor("yeT", [DT, P, C], F32, kind="ExternalOutput")
    ysT = nc.dram_tensor("ysT", [DT, P, TS], F32, kind="ExternalOutput")

    with tile.TileContext(nc) as tc:
        with (
            tc.tile_pool(name="xp", bufs=2) as xp,
            tc.tile_pool(name="wp", bufs=3) as wp,
            tc.tile_pool(name="w2p", bufs=3) as w2p,
            tc.tile_pool(name="atp", bufs=1) as atp,
            tc.tile_pool(name="op", bufs=2) as op,
            tc.tile_pool(name="ps", bufs=8, space="PSUM") as ps,
        ):
            pools = (xp, wp, w2p, atp, op, ps)
            aT_e = _emit_g1(nc, pools, xT, w13T, C, nch)
            # shared-expert x prefetch: lands during expert GEMM2
            xt_s = xp.tile([P, nchs, DK, CH], F16, tag="x", name="xt_s")
            for ci in range(nchs):
                nc.gpsimd.dma_start(out=xt_s[:, ci], in_=xsT[:][ci])
            _emit_g2(nc, pools, aT_e, w2T, yeT, C, nch)
            aT_s = _emit_g1(nc, pools, xsT, sw13T, TS, nchs, xt_pre=xt_s)
            _emit_g2(nc, pools, aT_s, sw2T, ysT, TS, nchs)
    _split_multiwaits(nc)
    return nc


_PROG_CACHE = {}

# test harnesses may override, e.g. {"trace": True, "trace_cores": [...]}
RUN_KWARGS = {}


def _get_program(C):
    if C not in _PROG_CACHE:
        _PROG_CACHE[C] = build_program(C)
    return _PROG_CACHE[C]


def _prep_w13(w):
    """w [2F', D] fp32 -> [FT', P, 2*DK*P] f16 in SBUF tile layout:
    out[ft, p, (h*DK + k)*P + fi] = w[h*F' + ft*P + fi, k*P + p]."""
    ft_n = w.shape[0] // (2 * P)
    a = w.astype(np.float16).reshape(2, ft_n, P, DK, P)  # (h, ft, fi, k, p)
    return np.ascontiguousarray(a.transpose(1, 4, 0, 3, 2)).reshape(
        ft_n, P, 2 * DK * P
    )


def _prep_w2(w):
    """w [D, F'] fp32 -> [DT, P, FT'*P] f16 in SBUF tile layout:
    out[dt, p, k*P + di] = w[dt*P + di, k*P + p]."""
    ft_n = w.shape[1] // P
    a = w.astype(np.float16).reshape(DT, P, ft_n, P)  # (dt, di, k, p)
    return np.ascontiguousarray(a.transpose(0, 3, 2, 1)).reshape(DT, P, ft_n * P)


def _prep_x(x16, C):
    """x16 [n<=C, D] f16 -> [C/CH, P, DK, CH] f16 zero-padded, chunk-major:
    out[ci, p, k, t] = x16[ci*CH + t, k*P + p]."""
    n = x16.shape[0]
    xe = np.zeros((C, D), np.float16)
    xe[:n] = x16
    a = xe.reshape(C // CH, CH, DK, P)  # (ci, t, k, p)
    return np.ascontiguousarray(a.transpose(0, 3, 2, 1))


def kernel(x, router_DE, w13, w2, shared_w13, shared_w2):
    x = np.asarray(x, dtype=np.float32)
    router_DE = np.asarray(router_DE, dtype=np.float32)
    w13 = np.asarray(w13, dtype=np.float32)
    w2 = np.asarray(w2, dtype=np.float32)
    shared_w13 = np.asarray(shared_w13, dtype=np.float32)
    shared_w2 = np.asarray(shared_w2, dtype=np.float32)

    # ---- routing (host) ----
    logits = x @ router_DE  # [T, E]
    top_idx = np.argsort(-logits, axis=1, kind="stable")[:, :TOP_K]  # [T, K]
    top_vals = np.take_along_axis(logits, top_idx, axis=1)
    ex = np.exp(top_vals - top_vals.max(axis=1, keepdims=True))
    gates = (ex / ex.sum(axis=1, keepdims=True)).astype(np.float32)

    toks_per_e, gates_per_e = [], []
    for e in range(E):
        hit = top_idx == e  # [T, K]
        tok_mask = hit.any(axis=1)
        toks = np.nonzero(tok_mask)[0]
        g = (gates * hit).sum(axis=1)[toks].astype(np.float32)
        toks_per_e.append(toks)
        gates_per_e.append(g)

    max_cnt = max(len(t) for t in toks_per_e)
    C = min(CAP, math.ceil(max_cnt / CH) * CH)

    # ---- host-side shard prep ----
    x16 = x.astype(np.float16)
    sw13T = _prep_w13(shared_w13)
    sw2T = _prep_w2(shared_w2)

    in_maps = []
    for c in range(NCORES):
        toks = toks_per_e[c]
        in_maps.append(
            {
                "xT": _prep_x(x16[toks[:C]], C),
                "w13T": _prep_w13(w13[c]),
                "w2T": _prep_w2(w2[c]),
                "xsT": _prep_x(x16[c * TS : (c + 1) * TS], TS),
                "sw13T": sw13T,
                "sw2T": sw2T,
            }
        )

    nc = _get_program(C)
    res = run_bass_kernel_spmd(nc, in_maps, list(range(NCORES)), **RUN_KWARGS)
    kernel.last_result = res

    # ---- combine (host) ----
    out = np.empty((T, D), np.float32)
    for c in range(NCORES):
        out[c * TS : (c + 1) * TS] = res.results[c]["ysT"].reshape(D, TS).T
    for c in range(NCORES):
        toks, g = toks_per_e[c], gates_per_e[c]
        cnt = min(len(toks), C)
        ye = res.results[c]["yeT"].reshape(D, C)
        out[toks[:cnt]] += (ye[:, :cnt] * g[:cnt][None, :]).T

    # ---- host fixup for tokens beyond the device capacity ----
    for e in range(E):
        toks, g = toks_per_e[e], gates_per_e[e]
        if len(toks) > C:
            to, go = toks[C:], g[C:]
            h = x[to] @ w13[e].T  # [n, 2F]
            a = (h[:, :F] / (1.0 + np.exp(-h[:, :F]))) * h[:, F:]
            out[to] += go[:, None] * (a @ w2[e].T)

    return out
